# revision 54
# baseline (speedup 1.0000x reference)
"""Fused transformer block (LN -> causal MHA -> proj -> LN -> FFN, residuals)
for trn2, 8 NeuronCores.

Sharding: core r handles batch b = r // 4 and head pair (2*(r%4), 2*(r%4)+1).
Each core runs exact-causal flash attention over the full 4096-token sequence
for its two heads, produces a partial projection output for the whole
sequence, ReduceScatters it over the 4 cores that share the batch (groups
[[0..3],[4..7]]), and finishes LN2 + FFN + residuals on its owned 1024 tokens.

Key perf structure vs the naive version:
  - scores run as fp8e4 DoubleRow matmuls (q/k quantized post-projection
    into a head-half-split [32,2] contraction layout), and att@V runs as
    fp8 DoubleRow over k-block pairs (p and V fp8) with the softmax
    denominators riding along as a ones-column in the V tile.  Final
    rel-err ~7e-3 vs the 2e-2 gate (fp8 on these operands is cheap: the
    softmax normalizes p, and score perturbations vanish under exp scale).
  - ACT runs only the softmax exp stream plus Ln/Exp-based rstd (same ACT
    table set as exp -> no 2.7us table reloads); SBUF-only squares/relu sit
    on Pool, PSUM reads on DVE (GPSIMD cannot touch PSUM).
  - LN1/QKV ("A/B") work and stage-D (LN2+FFN) work are split into small
    units drip-fed one-per-attention-pair into the emission stream, so the
    FIFO engine queues never hold a block long enough to starve the exp
    pipeline.
  - the proj ReduceScatter is split into 3 pieces (2048/1536/512 rows)
    fired as soon as their q-chunks complete, so only an ~18us collective
    plus one eighth of the FFN trails the last softmax.
"""
import os
import sys

sys.path.insert(0, "/opt/trn_rl_repo")

import numpy as np
import concourse.bass as bass
import concourse.mybir as mybir
from concourse import tile

F32 = mybir.dt.float32
BF16 = mybir.dt.bfloat16
F8 = mybir.dt.float8e4
NPBF16 = mybir.dt.np(BF16)
NPF8 = mybir.dt.np(F8)
DR = mybir.MatmulPerfMode.DoubleRow

B, T, C, H = 2, 4096, 512, 8
HS = C // H          # 64
FF = 4 * C           # 2048
EPS = 1e-5
SCL = float(C) ** -0.5
N_CORES = 8
GROUPS = [[0, 1, 2, 3], [4, 5, 6, 7]]
TSL = T // 4         # tokens owned per core after RS = 1024
NQC = 8              # q-chunks of 512
QC = T // NQC        # 512
AF = mybir.ActivationFunctionType

# ReduceScatter pieces: (global_row_start, global_rows). A big early piece,
# a mid piece, and a small tail piece so only an 18us collective plus one
# eighth of the FFN trail the last softmax.
RS_PIECES = [(0, 2048), (2048, 1536), (3584, 512)]
# stage-D token blocks (of 128 owned rows) per RS piece
PIECE_TBS = [[0, 1, 2, 3], [4, 5, 6], [7]]


def owned_slices(s):
    """Global (start, len) row slices owned by group-rank s, in local order."""
    out = []
    for g0, rows in RS_PIECES:
        ln = rows // 4
        out.append((g0 + ln * s, ln))
    return out


def assemble(per_core_outs):
    out = np.empty((B, T, C), np.float32)
    for r in range(N_CORES):
        b, s = r // 4, r % 4
        o = per_core_outs[r]
        loc = 0
        for g0, ln in owned_slices(s):
            out[b, g0:g0 + ln] = o[loc:loc + ln]
            loc += ln
    return out


def split_multiwaits(nc):
    """This toolchain's walrus accepts at most one sync-wait per instruction;
    Tile emits several.  Split extras into standalone EventSemaphore waits."""
    for fn in nc.m.functions:
        blocks = fn.blocks
        for blk in blocks:
            insts = blk.instructions
            new = []
            changed = False
            for inst in insts:
                si = inst.sync_info
                ows = list(si.on_wait) if si is not None else []
                if len(ows) > 1:
                    changed = True
                    for j, w in enumerate(ows[:-1]):
                        new.append(mybir.InstEventSemaphore(
                            name=f"{inst.name}_sw{j}",
                            engine=inst.engine,
                            ins=[], outs=[],
                            sync_info=mybir.SyncInfo(on_wait=[w], on_update=[]),
                        ))
                    inst.sync_info = mybir.SyncInfo(
                        on_wait=[ows[-1]], on_update=list(si.on_update))
                new.append(inst)
            if changed:
                blk.instructions = new
        fn.blocks = blocks


def build_nc(qkv_bias: bool, w1_bias: bool, has_bo: bool, has_b2: bool):
    nc = bass.Bass("TRN2", num_devices=N_CORES)

    # ---- DRAM I/O (per-core contents supplied by the host) ----
    xT_d = nc.dram_tensor("xT", [C, T], BF16, kind="ExternalInput")
    xsl_d = nc.dram_tensor("x_sl", [TSL, C], F32, kind="ExternalInput")
    wq2_d = nc.dram_tensor("wq2", [C + 1, 2 * HS], BF16, kind="ExternalInput")
    wk2_d = nc.dram_tensor("wk2", [C + 1, 2 * HS], BF16, kind="ExternalInput")
    wv2_d = nc.dram_tensor("wv2", [C + 1, 2 * HS], BF16, kind="ExternalInput")
    wo2_d = nc.dram_tensor("wo2", [2 * HS, C], BF16, kind="ExternalInput")
    w1_d = nc.dram_tensor("w1f", [C + 1, FF], BF16, kind="ExternalInput")
    w2_d = nc.dram_tensor("w2f", [FF, C], BF16, kind="ExternalInput")
    bias_d = nc.dram_tensor("biasv", [2, C], F32, kind="ExternalInput")
    tri_d = nc.dram_tensor("trimask", [4, 128, 2 * QC], BF16, kind="ExternalInput")
    ident_d = nc.dram_tensor("identm", [128, 128], BF16, kind="ExternalInput")

    proj_d = [nc.dram_tensor(f"proj_part{i}", [rows, C], BF16)
              for i, (_, rows) in enumerate(RS_PIECES)]
    rs_d = [nc.dram_tensor(f"proj_rs{i}", [rows // 4, C], BF16)
            for i, (_, rows) in enumerate(RS_PIECES)]
    out_d = nc.dram_tensor("out", [TSL, C], F32, kind="ExternalOutput")

    reps = int(os.environ.get("K_REPS", "1"))
    with tile.TileContext(nc) as tc:
        for rep in range(reps):
            _build_body(nc, tc, locals(), qkv_bias, w1_bias, has_bo, has_b2,
                        fence_off=rep * float(os.environ.get("K_ROFF", "0.36")))
    if not os.environ.get("K_NOSPLIT"):
        split_multiwaits(nc)
    return nc


def _build_body(nc, tc, d, qkv_bias, w1_bias, has_bo, has_b2, fence_off=0.0):
    xT_d, xsl_d = d["xT_d"], d["xsl_d"]
    wq2_d, wk2_d, wv2_d, wo2_d = d["wq2_d"], d["wk2_d"], d["wv2_d"], d["wo2_d"]
    w1_d, w2_d, bias_d, tri_d = d["w1_d"], d["w2_d"], d["bias_d"], d["tri_d"]
    ident_d = d["ident_d"]
    proj_d, rs_d, out_d = d["proj_d"], d["rs_d"], d["out_d"]

    def cc_rs(piece):
        nc.gpsimd.collective_compute(
            "ReduceScatter", mybir.AluOpType.add,
            ins=[proj_d[piece][:]], outs=[rs_d[piece][:]],
            replica_groups=GROUPS)

    import contextlib
    ctx = contextlib.ExitStack()
    with ctx:
        sing = ctx.enter_context(tc.tile_pool(name="sing", bufs=1))
        # one shared PSUM pool; tags partition the 8 banks:
        #   "sc" [128,1024]f32 x2  (4 banks: attention scores double-buffer)
        #   "s1" [128,512]f32 x2   (2 banks: stats/qkv/proj/ffn rotate)
        #   "o0"/"o1" [65,512] x1  (attention accumulators, 2 banks)
        ps = ctx.enter_context(tc.tile_pool(name="ps", bufs=2, space="PSUM"))
        pa = ctx.enter_context(tc.tile_pool(name="pa", bufs=3))
        pc_p = ctx.enter_context(tc.tile_pool(name="pc_p", bufs=3))
        pc_r = ctx.enter_context(tc.tile_pool(name="pc_r", bufs=2))
        pd = ctx.enter_context(tc.tile_pool(name="pd", bufs=2))
        pf = ctx.enter_context(tc.tile_pool(name="pf", bufs=1))

        # ---- persistent SBUF state ----
        # q/k in fp8, head-half-split layout for DoubleRow scores:
        # partitions [32h:32h+32] = head h dims [0:32] (j=0) / [32:64] (j=1)
        qT8 = sing.tile([64, 2, T], F8, tag="qT8", name="qT8")
        kT8 = sing.tile([64, 2, T], F8, tag="kT8", name="kT8")
        # fp8 V, k-block stride 80 (DoubleRow Ko step must be %16==0), with a
        # ones column at 64 accumulating the softmax denominator
        vst = [sing.tile([128, 32, 80], F8, tag=f"vst{h}", name=f"vst{h}")
               for h in range(2)]
        x2t = [sing.tile([128, C], F32, tag=f"x2t{tb}", name=f"x2t{tb}") for tb in range(8)]
        h2T = [sing.tile([128, TSL], BF16, tag=f"h2T{cb}", name=f"h2T{cb}") for cb in range(4)]

        # ---- constants ----
        ones1 = sing.tile([1, QC], BF16, tag="ones1", name="ones1")
        nc.vector.memset(ones1, 1.0)
        onesb = sing.tile([128, 128], BF16, tag="onesb", name="onesb")
        nc.vector.memset(onesb, 1.0 / C)
        ident = sing.tile([128, 128], BF16, tag="ident", name="ident")
        ones64f = sing.tile([1, HS], F32, tag="ones64f", name="ones64f")
        nc.vector.memset(ones64f, 1.0)
        epsb = sing.tile([128, 1], F32, tag="epsb", name="epsb")
        nc.vector.memset(epsb, EPS)

        # qkv weights: needed by the very first A/B chunk -- load first
        wqs = [sing.tile([128, 2 * HS], BF16, tag=f"wqs{cb}", name=f"wqs{cb}") for cb in range(4)]
        wks = [sing.tile([128, 2 * HS], BF16, tag=f"wks{cb}", name=f"wks{cb}") for cb in range(4)]
        wvs = [sing.tile([128, 2 * HS], BF16, tag=f"wvs{cb}", name=f"wvs{cb}") for cb in range(4)]
        if qkv_bias:
            wqb = sing.tile([1, 2 * HS], BF16, tag="wqb", name="wqb")
            wkb = sing.tile([1, 2 * HS], BF16, tag="wkb", name="wkb")
            wvb = sing.tile([1, 2 * HS], BF16, tag="wvb", name="wvb")
            nc.sync.dma_start(out=wqb, in_=wq2_d[C:C + 1, :])
            nc.sync.dma_start(out=wkb, in_=wk2_d[C:C + 1, :])
            nc.sync.dma_start(out=wvb, in_=wv2_d[C:C + 1, :])
        wo_sb = sing.tile([2 * HS, C], BF16, tag="wo", name="wo")
        tri = [sing.tile([128, 2 * QC], BF16, tag=f"tri{j}", name=f"tri{j}") for j in range(4)]
        w1sb = [sing.tile([128, FF], BF16, tag=f"w1s{cb}", name=f"w1s{cb}") for cb in range(4)]
        w2sb = [sing.tile([128, C], BF16, tag=f"w2s{hb}", name=f"w2s{hb}") for hb in range(16)]
        if w1_bias:
            w1b = sing.tile([1, FF], BF16, tag="w1b", name="w1b")

        def dram_row_bcast(dst, row_ap):
            src_ap = bass.AP(tensor=row_ap.tensor, offset=row_ap.offset,
                             ap=[[0, 128], [1, C]])
            nc.sync.dma_start(out=dst, in_=src_ap)

        if has_bo:
            bob = sing.tile([128, C], F32, tag="bob", name="bob")
        if has_b2:
            b2b = sing.tile([128, C], F32, tag="b2b", name="b2b")
        for h in range(2):
            nc.vector.memset(vst[h][:, :, HS:HS + 1], 1.0)

        # ========== stage A/B: LN1 -> hT; q/k (transposed), V (natural) ====
        # split into 3 units (LN stats+apply / q+k / v) so the PE work can be
        # drip-fed between attention pairs without starving the exp stream
        def ab_units(tch):
            sl = slice(tch * QC, (tch + 1) * QC)
            st = {}

            def u_ln():
                hT = [pa.tile([128, QC], BF16, tag=f"hT{cb}", name=f"hT{cb}")
                      for cb in range(4)]
                st["hT"] = hT
                if tch in xpre:
                    xb = xpre[tch]
                else:
                    xb = [pa.tile([128, QC], BF16, tag=f"xb{cb}", name=f"xb{cb}")
                          for cb in range(4)]
                    for cb in range(4):
                        nc.sync.dma_start(out=xb[cb],
                                          in_=xT_d[cb * 128:(cb + 1) * 128, sl])
                sq = [pa.tile([128, QC], BF16, tag=f"sq{cb}", name=f"sq{cb}")
                      for cb in range(4)]
                for cb in range(4):
                    nc.gpsimd.tensor_mul(sq[cb], xb[cb], xb[cb])
                mu_ps = ps.tile([128, QC], F32, tag="s1", name="mu_ps", bufs=2)
                sq_ps = ps.tile([128, QC], F32, tag="s1", name="sq_ps", bufs=2)
                for cb in range(4):
                    nc.tensor.matmul(mu_ps, onesb, xb[cb], start=(cb == 0), stop=(cb == 3))
                for cb in range(4):
                    nc.tensor.matmul(sq_ps, onesb, sq[cb], start=(cb == 0), stop=(cb == 3))
                mu_sb = pa.tile([128, QC], BF16, tag="mu_sb", name="mu_sb")
                nc.vector.tensor_copy(mu_sb, mu_ps)
                musq = pa.tile([128, QC], BF16, tag="musq", name="musq")
                nc.gpsimd.tensor_mul(musq, mu_sb, mu_sb)
                stat = pa.tile([128, QC], F32, tag="stat", name="stat")
                nc.vector.tensor_sub(stat, sq_ps, musq)
                # rstd = exp(-0.5*ln(var+eps)): Ln and Exp share one ACT
                # table set with the softmax exps -> no table reloads
                nc.scalar.activation(stat, stat, AF.Ln, bias=epsb)
                rstdb = pa.tile([128, QC], BF16, tag="rstdb", name="rstdb")
                with nc.allow_low_precision(reason="rstd in bf16"):
                    nc.scalar.activation(rstdb, stat, AF.Exp, scale=-0.5)
                for cb in range(4):
                    nc.vector.tensor_sub(hT[cb], xb[cb], mu_sb)
                    nc.vector.tensor_mul(hT[cb], hT[cb], rstdb)

            def u_qk():
                hT = st["hT"]
                q_ps = ps.tile([128, QC], F32, tag="s1", name="q_ps", bufs=2)
                k_ps = ps.tile([128, QC], F32, tag="s1", name="k_ps", bufs=2)
                for cb in range(4):
                    nc.tensor.matmul(q_ps, wqs[cb], hT[cb],
                                     start=(cb == 0), stop=(cb == 3 and not qkv_bias))
                    nc.tensor.matmul(k_ps, wks[cb], hT[cb],
                                     start=(cb == 0), stop=(cb == 3 and not qkv_bias))
                if qkv_bias:
                    nc.tensor.matmul(q_ps, wqb, ones1, start=False, stop=True)
                    nc.tensor.matmul(k_ps, wkb, ones1, start=False, stop=True)
                # wq/wk columns are pre-permuted host-side so q_ps partitions
                # are [h0 lo | h1 lo | h0 hi | h1 hi]; two Pool copies (with a
                # partition shift on the upper half) land the DoubleRow layout
                nc.vector.tensor_copy(qT8[:, 0, sl], q_ps[0:64, :])
                nc.vector.tensor_copy(qT8[:, 1, sl], q_ps[64:128, :])
                nc.vector.tensor_copy(kT8[:, 0, sl], k_ps[0:64, :])
                nc.vector.tensor_copy(kT8[:, 1, sl], k_ps[64:128, :])

            def u_v():
                hT = st["hT"]
                for t4 in range(4):
                    tb = tch * 4 + t4
                    t4sl = slice(t4 * 128, (t4 + 1) * 128)
                    v_ps = ps.tile([128, 2 * HS], F32, tag="s1", name="v_ps", bufs=2)
                    for cb in range(4):
                        nc.tensor.matmul(v_ps, hT[cb][:, t4sl], wvs[cb],
                                         start=(cb == 0), stop=(cb == 3 and not qkv_bias))
                    if qkv_bias:
                        nc.tensor.matmul(v_ps, ones1[:, 0:128], wvb, start=False, stop=True)
                    for h in range(2):
                        nc.vector.tensor_copy(vst[h][:, tb, 0:HS],
                                              v_ps[:, h * HS:(h + 1) * HS])

            return [u_ln, u_qk, u_v]

        def emit_ab(tch):
            for u in ab_units(tch):
                u()

        # ========== stage C: flash attention + packed proj partial =========
        abq = []     # pending A/B units: highest priority, one per pair
        gq = []      # weight loads + stage-D units, one per pair otherwise
        tailq = []   # previous chunk's renorm/proj units: must run first so
                     # o_ps frees for this chunk's att@V accumulation

        def emit_c(c):
            qsl = slice(c * QC, (c + 1) * QC)
            nkv = 4 * (c + 1)
            npair = nkv // 2
            o_ps = [ps.tile([65, QC], F32, tag=f"o{h}", name=f"o{h}", bufs=1)
                    for h in range(2)]
            for pi in range(npair):
                if pi != 0:
                    if tailq:
                        tailq.pop(0)()
                    elif abq:
                        abq.pop(0)()
                    elif gq:
                        gq.pop(0)()
                p4 = pc_p.tile([128, 2, 2 * QC], F8, tag="p4", name="p4")
                for j2 in range(2):
                    kb = 2 * pi + j2
                    ksl = slice(kb * 128, (kb + 1) * 128)
                    diag = kb >= 4 * c
                    s_ps = ps.tile([128, 2 * QC], F32, tag="sc", name="sc", bufs=2)
                    for h in range(2):
                        hp = slice(h * 32, (h + 1) * 32)
                        nc.tensor.matmul(s_ps[:, h * QC:(h + 1) * QC],
                                         kT8[hp, :, ksl], qT8[hp, :, qsl],
                                         start=True, stop=not diag,
                                         perf_mode=DR)
                    if diag:
                        # additive causal mask (-1e6 above diagonal) folded
                        # into the PSUM accumulation: s += I^T @ mneg_j
                        j = kb - 4 * c
                        nc.tensor.matmul(s_ps[:, 0:QC], ident, tri[j][:, 0:QC],
                                         start=False, stop=True)
                        nc.tensor.matmul(s_ps[:, QC:2 * QC], ident, tri[j][:, QC:2 * QC],
                                         start=False, stop=True)
                    nc.scalar.activation(p4[:, j2, :], s_ps, AF.Exp, scale=SCL)
                if pi == 0 and tailq:
                    # previous chunk's renorm slots in between pair-0's
                    # scores/exp and its att@V, so this chunk's first av
                    # (which waits for o_ps to free) finds renorm done
                    tailq.pop(0)()
                # att @ V over the k-block pair in one fp8 DoubleRow matmul
                for h in range(2):
                    nc.tensor.matmul(o_ps[h], vst[h][:, 2 * pi:2 * pi + 2, 0:HS + 1],
                                     p4[:, :, h * QC:(h + 1) * QC],
                                     start=(pi == 0), stop=(pi == npair - 1),
                                     perf_mode=DR)
            # renorm + proj are deferred into the NEXT chunk's pair stream
            # (tailq) so their DVE->PE chain latency hides under its exps
            def t_renorm():
                on_t = pc_r.tile([128, QC], BF16, tag="on", name="on")
                for h in range(2):
                    rd = pc_r.tile([1, QC], F32, tag=f"rd{h}", name=f"rd{h}")
                    nc.vector.reciprocal(rd, o_ps[h][64:65, :])
                    rb_ps = ps.tile([64, QC], F32, tag="s1", name="rb_ps", bufs=2)
                    nc.tensor.matmul(rb_ps, ones64f, rd, start=True, stop=True)
                    rb = pc_r.tile([64, QC], F32, tag=f"rb{h}", name=f"rb{h}")
                    nc.vector.tensor_copy(rb, rb_ps)
                    nc.vector.tensor_mul(on_t[h * 64:(h + 1) * 64, :],
                                         o_ps[h][0:64, :], rb)
                st_c["on_t"] = on_t

            def t_proj():
                on_t = st_c["on_t"]
                for t4 in range(4):
                    t4sl = slice(t4 * 128, (t4 + 1) * 128)
                    pp = ps.tile([128, C], F32, tag="s1", name="pp", bufs=2)
                    nc.tensor.matmul(pp, on_t[:, t4sl], wo_sb, start=True, stop=True)
                    ppsb = pc_p.tile([128, C], BF16, tag="ppsb", name="ppsb")
                    nc.vector.tensor_copy(ppsb, pp)
                    gr = c * QC + t4 * 128
                    piece = 0 if gr < 2048 else (1 if gr < 3584 else 2)
                    r0 = gr - RS_PIECES[piece][0]
                    nc.gpsimd.dma_start(out=proj_d[piece][r0:r0 + 128, :], in_=ppsb)

            st_c = {}
            t_renorm()
            t_proj()

        # ========== stage D: LN2 -> FFN -> out for one RS piece ============
        # split into small "units" so the work can be drip-fed between
        # attention pairs (the PE/DVE queues are FIFO: one big block of
        # stage-D instructions would stall the next chunk's scores and
        # starve the exp stream)
        d_f1 = {}

        def d_ln_tb(piece, tb):
            tsl = slice(tb * 128, (tb + 1) * 128)
            rstb = pd.tile([128, C], BF16, tag="rstb", name="rstb")
            hr = (tb - PIECE_TBS[piece][0]) * 128
            nc.sync.dma_start(out=rstb, in_=rs_d[piece][hr:hr + 128, :])
            xst = pd.tile([128, C], F32, tag="xst", name="xst")
            nc.sync.dma_start(out=xst, in_=xsl_d[tsl, :])
            nc.vector.tensor_add(x2t[tb], rstb, xst)
            if has_bo:
                nc.vector.tensor_add(x2t[tb], x2t[tb], bob)
            st6 = pd.tile([128, 6], F32, tag="st6", name="st6")
            nc.vector.bn_stats(st6, x2t[tb])
            mv = pd.tile([128, 2], F32, tag="mv", name="mv")
            nc.vector.bn_aggr(mv, st6)
            vv = pd.tile([128, 1], F32, tag="vv", name="vv")
            nc.scalar.activation(vv, mv[:, 1:2], AF.Ln, bias=epsb)
            rs2 = pd.tile([128, 1], F32, tag="rs2", name="rs2")
            nc.scalar.activation(rs2, vv, AF.Exp, scale=-0.5)
            h2 = pd.tile([128, C], BF16, tag="h2", name="h2")
            nc.vector.tensor_scalar(h2, x2t[tb], mv[:, 0:1], rs2,
                                    op0=mybir.AluOpType.subtract,
                                    op1=mybir.AluOpType.mult)
            for cb in range(4):
                tp = ps.tile([128, 128], BF16, tag="s1", name="tp", bufs=2)
                nc.tensor.transpose(tp, h2[:, cb * 128:(cb + 1) * 128], ident)
                nc.vector.tensor_copy(h2T[cb][:, tsl], tp)

        def d_w1_hb(grp, hb):
            nt = len(grp)
            csl = slice(grp[0] * 128, (grp[0] + nt) * 128)
            f1 = pf.tile([128, 2 * 128], BF16, tag=f"f1_{hb}", name=f"f1_{hb}")
            d_f1[(grp[0], hb)] = f1
            hsl = slice(hb * 128, (hb + 1) * 128)
            f_ps = ps.tile([128, nt * 128], F32, tag="s1", name="f_ps", bufs=2)
            for cb in range(4):
                nc.tensor.matmul(f_ps, w1sb[cb][:, hsl], h2T[cb][:, csl],
                                 start=(cb == 0), stop=(cb == 3 and not w1_bias))
            if w1_bias:
                nc.tensor.matmul(f_ps, w1b[:, hsl], ones1[:, 0:nt * 128],
                                 start=False, stop=True)
            nc.vector.tensor_scalar_max(f1[:, 0:nt * 128], f_ps, 0.0)

        def d_w2_tb(grp, tb):
            i = tb - grp[0]
            tsl = slice(tb * 128, (tb + 1) * 128)
            fsl = slice(i * 128, (i + 1) * 128)
            ff_ps = ps.tile([128, C], F32, tag="s1", name="ff_ps", bufs=2)
            for hb in range(16):
                nc.tensor.matmul(ff_ps, d_f1[(grp[0], hb)][:, fsl], w2sb[hb],
                                 start=(hb == 0), stop=(hb == 15))
            ot = pd.tile([128, C], F32, tag="ot", name="ot")
            nc.vector.tensor_add(ot, ff_ps, x2t[tb])
            if has_b2:
                nc.vector.tensor_add(ot, ot, b2b)
            nc.sync.dma_start(out=out_d[tsl, :], in_=ot)

        def d_units(piece):
            # FFN runs per 2-tb group to bound the f1 SBUF footprint
            units = []
            tbs = PIECE_TBS[piece]
            for tb in tbs:
                units.append(lambda tb=tb: d_ln_tb(piece, tb))
            for g0 in range(0, len(tbs), 2):
                grp = tbs[g0:g0 + 2]
                for hb in range(16):
                    units.append(lambda hb=hb, grp=tuple(grp): d_w1_hb(grp, hb))
                for tb in grp:
                    units.append(lambda tb=tb, grp=tuple(grp): d_w2_tb(grp, tb))
            return units

        # ========== emission schedule ==========
        # chunk-0/1 x loads configure on the SP sequencer first (565ns per
        # DMA config); the qkv weights follow, still landing before the
        # first LN1 output reaches the q/k/v matmuls
        # startup DMA order tracks the first-exp critical chain:
        # x0 -> stats -> LN -> q/k -> scores(kb0, diagonal needs tri)
        xpre = {}
        for tch in (0, 1):
            xpre[tch] = [pa.tile([128, QC], BF16, tag=f"xb{cb}", name=f"xb{cb}")
                         for cb in range(4)]
        # chunks 4-7's x tiles persist so their loads can be issued before
        # the first ReduceScatter blocks the SP queue
        for tch in range(4, NQC):
            xpre[tch] = [sing.tile([128, QC], BF16, tag=f"xp{tch}_{cb}",
                                   name=f"xp{tch}_{cb}") for cb in range(4)]
        for cb in range(4):
            nc.sync.dma_start(out=xpre[0][cb], in_=xT_d[cb * 128:(cb + 1) * 128, 0:QC])
        for cb in range(4):
            nc.sync.dma_start(out=wqs[cb], in_=wq2_d[cb * 128:(cb + 1) * 128, :])
            nc.sync.dma_start(out=wks[cb], in_=wk2_d[cb * 128:(cb + 1) * 128, :])
        nc.sync.dma_start(out=tri[0], in_=tri_d[0])
        nc.sync.dma_start(out=ident, in_=ident_d[:])
        for j in range(1, 4):
            nc.sync.dma_start(out=tri[j], in_=tri_d[j])
        for cb in range(4):
            nc.sync.dma_start(out=xpre[1][cb], in_=xT_d[cb * 128:(cb + 1) * 128, QC:2 * QC])
        for cb in range(4):
            nc.sync.dma_start(out=wvs[cb], in_=wv2_d[cb * 128:(cb + 1) * 128, :])
        nc.sync.dma_start(out=wo_sb, in_=wo2_d[:])
        emit_ab(0)
        emit_ab(1)
        if has_bo:
            dram_row_bcast(bob, bias_d[0:1, :])
        if has_b2:
            dram_row_bcast(b2b, bias_d[1:2, :])
        # heavy FFN weights: deferred into the pair stream via gq so their
        # DMA configs never crowd the SP queue ahead of attention loads
        wq_port = {"sp": nc.sync, "act": nc.scalar, "pool": nc.gpsimd,
                   "dve": nc.vector}[os.environ.get("K_WQ", "sp")]

        def wload(dst, src):
            return lambda: wq_port.dma_start(out=dst, in_=src)

        for cb in range(4):
            gq.append(wload(w1sb[cb], w1_d[cb * 128:(cb + 1) * 128, :]))
        if w1_bias:
            gq.append(wload(w1b, w1_d[C:C + 1, :]))
        for hb in range(16):
            gq.append(wload(w2sb[hb], w2_d[hb * 128:(hb + 1) * 128, :]))

        # in-order q-chunks.  A/B units for chunk c+2 and stage-D units for
        # already-reduced RS pieces are drip-fed one per attention pair, so
        # the FIFO engine queues never hold a long block that would starve
        # the exp stream.  RS piece 0 (rows 0-2047) fires after C3, piece 1
        # (2048-3583) after C6, the small tail piece after C7.
        for c in range(NQC):
            if c + 2 < NQC:
                abq.extend(ab_units(c + 2))
            emit_c(c)
            if c == 1:
                for tch in range(4, NQC):
                    sl = slice(tch * QC, (tch + 1) * QC)
                    for cb in range(4):
                        nc.sync.dma_start(out=xpre[tch][cb],
                                          in_=xT_d[cb * 128:(cb + 1) * 128, sl])
            elif c == 3:
                # flush this chunk's renorm/proj so the RS (whose deps are
                # tracked by emission order) can be emitted now
                while tailq:
                    tailq.pop(0)()
                cc_rs(0)
            elif c == 4:
                # piece-0 D work becomes dependency-ready at ~82us (RS0
                # done); feed it into the C5/C6 pair stream
                gq.extend(d_units(0))
            elif c == 6:
                while tailq:
                    tailq.pop(0)()
                cc_rs(1)
        # leftover piece-0 units plus all of piece 1: piece 1's D runs
        # during the tail collective's window
        while tailq:
            tailq.pop(0)()
        for u in gq:
            u()
        gq.clear()
        for u in d_units(1):
            u()
        cc_rs(2)
        with tc.tile_wait_until(fence_off + float(os.environ.get("K_F2", "0.180"))):
            for u in d_units(2):
                u()


_NC_CACHE = {}


def _get_nc(flags):
    if flags not in _NC_CACHE:
        _NC_CACHE[flags] = build_nc(*flags)
    return _NC_CACHE[flags]


def make_in_maps(x, wq, wk, wv, wo, bo, w1, b1, w2, b2, g1, be1, g2, be2):
    x = np.asarray(x, np.float32)
    f32 = lambda a: np.ascontiguousarray(np.asarray(a, np.float32))
    wq, wk, wv, wo, w1, w2 = map(f32, (wq, wk, wv, wo, w1, w2))
    bo, b1, b2, g1, be1, g2, be2 = map(f32, (bo, b1, b2, g1, be1, g2, be2))

    # fold LN affine into the consuming matmuls
    wq_s = g1[None, :, None] * wq            # [H, C, HS]
    wk_s = g1[None, :, None] * wk
    wv_s = g1[None, :, None] * wv
    cq = np.einsum("c,hcd->hd", be1, wq)     # [H, HS]
    ck = np.einsum("c,hcd->hd", be1, wk)
    cv = np.einsum("c,hcd->hd", be1, wv)
    w1_s = g2[:, None] * w1                  # [C, FF]
    c1 = b1 + be2 @ w1                       # [FF]

    qkv_bias = bool(np.any(cq) or np.any(ck) or np.any(cv))
    w1_bias = bool(np.any(c1))
    has_bo = bool(np.any(bo))
    has_b2 = bool(np.any(b2))
    flags = (qkv_bias, w1_bias, has_bo, has_b2)

    w1f = np.concatenate([w1_s, c1[None, :]], 0).astype(NPBF16)   # [513, FF]
    w2f = w2.astype(NPBF16)
    biasv = np.stack([bo, b2]).astype(np.float32)                 # [2, C]

    # causal masks (additive, pre-exp) for the 4 diagonal 128-k blocks of a
    # q-chunk, in S^T layout, doubled for the 2-head tiles
    k_in = np.arange(128)[:, None]
    q_in = np.arange(QC)[None, :]
    tri = np.stack([
        np.where(j * 128 + k_in <= q_in, 0.0, -1.0e6) for j in range(4)
    ])
    tri = np.tile(tri, (1, 1, 2)).astype(NPBF16)                  # [4, 128, 2*QC]

    in_maps = []
    for r in range(N_CORES):
        b, hp = r // 4, r % 4
        h0 = 2 * hp
        s = r % 4
        # q/k columns permuted to [h0 lo32 | h1 lo32 | h0 hi32 | h1 hi32] so
        # q_ps/k_ps partitions match the DoubleRow scores layout directly
        def qperm(w_h0, w_h1, c_h0, c_h1):
            wcat = np.concatenate([w_h0[:, :32], w_h1[:, :32],
                                   w_h0[:, 32:], w_h1[:, 32:]], 1)
            ccat = np.concatenate([c_h0[:32], c_h1[:32], c_h0[32:], c_h1[32:]])
            return np.concatenate([wcat, ccat[None, :]], 0).astype(NPBF16)

        wq2 = qperm(wq_s[h0], wq_s[h0 + 1], cq[h0], cq[h0 + 1])
        wk2 = qperm(wk_s[h0], wk_s[h0 + 1], ck[h0], ck[h0 + 1])
        wv2 = np.concatenate([
            np.concatenate([wv_s[h0], wv_s[h0 + 1]], 1),
            np.concatenate([cv[h0], cv[h0 + 1]])[None, :]], 0).astype(NPBF16)
        wo2 = wo[h0 * HS:(h0 + 2) * HS, :].astype(NPBF16)         # [128, C]
        x_sl = np.concatenate(
            [x[b, g0:g0 + ln] for g0, ln in owned_slices(s)], 0)
        in_maps.append({
            "xT": np.ascontiguousarray(x[b].T).astype(NPBF16),
            "x_sl": np.ascontiguousarray(x_sl),
            "wq2": wq2, "wk2": wk2, "wv2": wv2, "wo2": wo2,
            "w1f": w1f, "w2f": w2f, "biasv": biasv, "trimask": tri,
            "identm": np.eye(128, dtype=np.float32).astype(NPBF16),
        })
    return in_maps, flags


def run_spmd(in_maps, flags, **kw):
    from concourse.bass_utils import run_bass_kernel_spmd
    nc = _get_nc(flags)
    return run_bass_kernel_spmd(nc, in_maps, list(range(N_CORES)), **kw)


def kernel(**inputs):
    in_maps, flags = make_in_maps(**inputs)
    res = run_spmd(in_maps, flags).results
    return assemble([res[r]["out"] for r in range(N_CORES)])


# revision 56
# speedup vs baseline: 5.2372x; 5.2372x over previous
"""Fused transformer block (LN -> causal MHA -> proj -> LN -> FFN, residuals)
for trn2, 8 NeuronCores.

Sharding: core r handles batch b = r // 4 and head pair (2*(r%4), 2*(r%4)+1).
Each core runs exact-causal flash attention over the full 4096-token sequence
for its two heads, produces a partial projection output for the whole
sequence, ReduceScatters it over the 4 cores that share the batch (groups
[[0..3],[4..7]]), and finishes LN2 + FFN + residuals on its owned 1024 tokens.

Key perf structure vs the naive version:
  - scores run as fp8e4 DoubleRow matmuls (q/k quantized post-projection
    into a head-half-split [32,2] contraction layout), and att@V runs as
    fp8 DoubleRow over k-block pairs (p and V fp8) with the softmax
    denominators riding along as a ones-column in the V tile.  Final
    rel-err ~7e-3 vs the 2e-2 gate (fp8 on these operands is cheap: the
    softmax normalizes p, and score perturbations vanish under exp scale).
  - ACT runs only the softmax exp stream plus Ln/Exp-based rstd (same ACT
    table set as exp -> no 2.7us table reloads); SBUF-only squares/relu sit
    on Pool, PSUM reads on DVE (GPSIMD cannot touch PSUM).
  - LN1/QKV ("A/B") work and stage-D (LN2+FFN) work are split into small
    units drip-fed one-per-attention-pair into the emission stream, so the
    FIFO engine queues never hold a block long enough to starve the exp
    pipeline.
  - the proj ReduceScatter is split into 3 pieces (2048/1536/512 rows)
    fired as soon as their q-chunks complete, so only an ~18us collective
    plus one eighth of the FFN trails the last softmax.
"""
import os
import sys

sys.path.insert(0, "/opt/trn_rl_repo")

import numpy as np
import concourse.bass as bass
import concourse.mybir as mybir
from concourse import tile

F32 = mybir.dt.float32
BF16 = mybir.dt.bfloat16
F8 = mybir.dt.float8e4
NPBF16 = mybir.dt.np(BF16)
NPF8 = mybir.dt.np(F8)
DR = mybir.MatmulPerfMode.DoubleRow

B, T, C, H = 2, 4096, 512, 8
HS = C // H          # 64
FF = 4 * C           # 2048
EPS = 1e-5
SCL = float(C) ** -0.5
N_CORES = 8
GROUPS = [[0, 1, 2, 3], [4, 5, 6, 7]]
TSL = T // 4         # tokens owned per core after RS = 1024
NQC = 8              # q-chunks of 512
QC = T // NQC        # 512
AF = mybir.ActivationFunctionType

# ReduceScatter pieces: (global_row_start, global_rows). A big early piece,
# a mid piece, and a small tail piece so only an 18us collective plus one
# eighth of the FFN trail the last softmax.
RS_PIECES = [(0, 2048), (2048, 1536), (3584, 512)]
# stage-D token blocks (of 128 owned rows) per RS piece
PIECE_TBS = [[0, 1, 2, 3], [4, 5, 6], [7]]


def owned_slices(s):
    """Global (start, len) row slices owned by group-rank s, in local order."""
    out = []
    for g0, rows in RS_PIECES:
        ln = rows // 4
        out.append((g0 + ln * s, ln))
    return out


def assemble(per_core_outs):
    out = np.empty((B, T, C), np.float32)
    for r in range(N_CORES):
        b, s = r // 4, r % 4
        o = per_core_outs[r]
        loc = 0
        for g0, ln in owned_slices(s):
            out[b, g0:g0 + ln] = o[loc:loc + ln]
            loc += ln
    return out


def split_multiwaits(nc):
    """This toolchain's walrus accepts at most one sync-wait per instruction;
    Tile emits several.  Split extras into standalone EventSemaphore waits."""
    for fn in nc.m.functions:
        blocks = fn.blocks
        for blk in blocks:
            insts = blk.instructions
            new = []
            changed = False
            for inst in insts:
                si = inst.sync_info
                ows = list(si.on_wait) if si is not None else []
                if len(ows) > 1:
                    changed = True
                    for j, w in enumerate(ows[:-1]):
                        new.append(mybir.InstEventSemaphore(
                            name=f"{inst.name}_sw{j}",
                            engine=inst.engine,
                            ins=[], outs=[],
                            sync_info=mybir.SyncInfo(on_wait=[w], on_update=[]),
                        ))
                    inst.sync_info = mybir.SyncInfo(
                        on_wait=[ows[-1]], on_update=list(si.on_update))
                new.append(inst)
            if changed:
                blk.instructions = new
        fn.blocks = blocks


def build_nc(qkv_bias: bool, w1_bias: bool, has_bo: bool, has_b2: bool):
    nc = bass.Bass("TRN2", num_devices=N_CORES)

    # ---- DRAM I/O (per-core contents supplied by the host) ----
    xT_d = nc.dram_tensor("xT", [C, T], BF16, kind="ExternalInput")
    xsl_d = nc.dram_tensor("x_sl", [TSL, C], F32, kind="ExternalInput")
    wq2_d = nc.dram_tensor("wq2", [C + 1, 2 * HS], BF16, kind="ExternalInput")
    wk2_d = nc.dram_tensor("wk2", [C + 1, 2 * HS], BF16, kind="ExternalInput")
    wv2_d = nc.dram_tensor("wv2", [C + 1, 2 * HS], BF16, kind="ExternalInput")
    wo2_d = nc.dram_tensor("wo2", [2 * HS, C], BF16, kind="ExternalInput")
    w1_d = nc.dram_tensor("w1f", [C + 1, FF], BF16, kind="ExternalInput")
    w2_d = nc.dram_tensor("w2f", [FF, C], BF16, kind="ExternalInput")
    bias_d = nc.dram_tensor("biasv", [2, C], F32, kind="ExternalInput")
    tri_d = nc.dram_tensor("trimask", [4, 128, 2 * QC], BF16, kind="ExternalInput")
    ident_d = nc.dram_tensor("identm", [128, 128], BF16, kind="ExternalInput")

    proj_d = [nc.dram_tensor(f"proj_part{i}", [rows, C], BF16)
              for i, (_, rows) in enumerate(RS_PIECES)]
    rs_d = [nc.dram_tensor(f"proj_rs{i}", [rows // 4, C], BF16)
            for i, (_, rows) in enumerate(RS_PIECES)]
    out_d = nc.dram_tensor("out", [TSL, C], F32, kind="ExternalOutput")

    reps = int(os.environ.get("K_REPS", "1"))
    with tile.TileContext(nc) as tc:
        for rep in range(reps):
            _build_body(nc, tc, locals(), qkv_bias, w1_bias, has_bo, has_b2,
                        fence_off=rep * float(os.environ.get("K_ROFF", "0.36")))
    if not os.environ.get("K_NOSPLIT"):
        split_multiwaits(nc)
    return nc


def _build_body(nc, tc, d, qkv_bias, w1_bias, has_bo, has_b2, fence_off=0.0):
    xT_d, xsl_d = d["xT_d"], d["xsl_d"]
    wq2_d, wk2_d, wv2_d, wo2_d = d["wq2_d"], d["wk2_d"], d["wv2_d"], d["wo2_d"]
    w1_d, w2_d, bias_d, tri_d = d["w1_d"], d["w2_d"], d["bias_d"], d["tri_d"]
    ident_d = d["ident_d"]
    proj_d, rs_d, out_d = d["proj_d"], d["rs_d"], d["out_d"]

    def cc_rs(piece):
        nc.gpsimd.collective_compute(
            "ReduceScatter", mybir.AluOpType.add,
            ins=[proj_d[piece][:]], outs=[rs_d[piece][:]],
            replica_groups=GROUPS)

    import contextlib
    ctx = contextlib.ExitStack()
    with ctx:
        sing = ctx.enter_context(tc.tile_pool(name="sing", bufs=1))
        # one shared PSUM pool; tags partition the 8 banks:
        #   "sc" [128,1024]f32 x2  (4 banks: attention scores double-buffer)
        #   "s1" [128,512]f32 x2   (2 banks: stats/qkv/proj/ffn rotate)
        #   "o0"/"o1" [65,512] x1  (attention accumulators, 2 banks)
        ps = ctx.enter_context(tc.tile_pool(name="ps", bufs=2, space="PSUM"))
        pa = ctx.enter_context(tc.tile_pool(name="pa", bufs=3))
        pc_p = ctx.enter_context(tc.tile_pool(name="pc_p", bufs=3))
        pc_r = ctx.enter_context(tc.tile_pool(name="pc_r", bufs=2))
        pd = ctx.enter_context(tc.tile_pool(name="pd", bufs=2))
        pf = ctx.enter_context(tc.tile_pool(name="pf", bufs=1))

        # ---- persistent SBUF state ----
        # q/k in fp8, head-half-split layout for DoubleRow scores:
        # partitions [32h:32h+32] = head h dims [0:32] (j=0) / [32:64] (j=1)
        qT8 = sing.tile([64, 2, T], F8, tag="qT8", name="qT8")
        kT8 = sing.tile([64, 2, T], F8, tag="kT8", name="kT8")
        # fp8 V, k-block stride 80 (DoubleRow Ko step must be %16==0), with a
        # ones column at 64 accumulating the softmax denominator
        vst = [sing.tile([128, 32, 80], F8, tag=f"vst{h}", name=f"vst{h}")
               for h in range(2)]
        x2t = [sing.tile([128, C], F32, tag=f"x2t{tb}", name=f"x2t{tb}") for tb in range(8)]
        h2T = [sing.tile([128, TSL], BF16, tag=f"h2T{cb}", name=f"h2T{cb}") for cb in range(4)]

        # ---- constants ----
        ones1 = sing.tile([1, QC], BF16, tag="ones1", name="ones1")
        nc.vector.memset(ones1, 1.0)
        onesb = sing.tile([128, 128], BF16, tag="onesb", name="onesb")
        nc.vector.memset(onesb, 1.0 / C)
        ident = sing.tile([128, 128], BF16, tag="ident", name="ident")
        ones64f = sing.tile([1, HS], F32, tag="ones64f", name="ones64f")
        nc.vector.memset(ones64f, 1.0)
        epsb = sing.tile([128, 1], F32, tag="epsb", name="epsb")
        nc.vector.memset(epsb, EPS)

        # qkv weights: needed by the very first A/B chunk -- load first
        wqs = [sing.tile([128, 2 * HS], BF16, tag=f"wqs{cb}", name=f"wqs{cb}") for cb in range(4)]
        wks = [sing.tile([128, 2 * HS], BF16, tag=f"wks{cb}", name=f"wks{cb}") for cb in range(4)]
        wvs = [sing.tile([128, 2 * HS], BF16, tag=f"wvs{cb}", name=f"wvs{cb}") for cb in range(4)]
        if qkv_bias:
            wqb = sing.tile([1, 2 * HS], BF16, tag="wqb", name="wqb")
            wkb = sing.tile([1, 2 * HS], BF16, tag="wkb", name="wkb")
            wvb = sing.tile([1, 2 * HS], BF16, tag="wvb", name="wvb")
            nc.sync.dma_start(out=wqb, in_=wq2_d[C:C + 1, :])
            nc.sync.dma_start(out=wkb, in_=wk2_d[C:C + 1, :])
            nc.sync.dma_start(out=wvb, in_=wv2_d[C:C + 1, :])
        wo_sb = sing.tile([2 * HS, C], BF16, tag="wo", name="wo")
        tri = [sing.tile([128, 2 * QC], BF16, tag=f"tri{j}", name=f"tri{j}") for j in range(4)]
        w1sb = [sing.tile([128, FF], BF16, tag=f"w1s{cb}", name=f"w1s{cb}") for cb in range(4)]
        w2sb = [sing.tile([128, C], BF16, tag=f"w2s{hb}", name=f"w2s{hb}") for hb in range(16)]
        if w1_bias:
            w1b = sing.tile([1, FF], BF16, tag="w1b", name="w1b")

        def dram_row_bcast(dst, row_ap):
            src_ap = bass.AP(tensor=row_ap.tensor, offset=row_ap.offset,
                             ap=[[0, 128], [1, C]])
            nc.sync.dma_start(out=dst, in_=src_ap)

        if has_bo:
            bob = sing.tile([128, C], F32, tag="bob", name="bob")
        if has_b2:
            b2b = sing.tile([128, C], F32, tag="b2b", name="b2b")
        for h in range(2):
            nc.vector.memset(vst[h][:, :, HS:HS + 1], 1.0)

        # ========== stage A/B: LN1 -> hT; q/k (transposed), V (natural) ====
        # split into 3 units (LN stats+apply / q+k / v) so the PE work can be
        # drip-fed between attention pairs without starving the exp stream
        def ab_units(tch):
            sl = slice(tch * QC, (tch + 1) * QC)
            st = {}

            def u_ln():
                hT = [pa.tile([128, QC], BF16, tag=f"hT{cb}", name=f"hT{cb}")
                      for cb in range(4)]
                st["hT"] = hT
                if tch in xpre:
                    xb = xpre[tch]
                else:
                    xb = [pa.tile([128, QC], BF16, tag=f"xb{cb}", name=f"xb{cb}")
                          for cb in range(4)]
                    for cb in range(4):
                        nc.sync.dma_start(out=xb[cb],
                                          in_=xT_d[cb * 128:(cb + 1) * 128, sl])
                sq = [pa.tile([128, QC], BF16, tag=f"sq{cb}", name=f"sq{cb}")
                      for cb in range(4)]
                for cb in range(4):
                    nc.gpsimd.tensor_mul(sq[cb], xb[cb], xb[cb])
                mu_ps = ps.tile([128, QC], F32, tag="s1", name="mu_ps", bufs=2)
                sq_ps = ps.tile([128, QC], F32, tag="s1", name="sq_ps", bufs=2)
                for cb in range(4):
                    nc.tensor.matmul(mu_ps, onesb, xb[cb], start=(cb == 0), stop=(cb == 3))
                for cb in range(4):
                    nc.tensor.matmul(sq_ps, onesb, sq[cb], start=(cb == 0), stop=(cb == 3))
                mu_sb = pa.tile([128, QC], BF16, tag="mu_sb", name="mu_sb")
                nc.vector.tensor_copy(mu_sb, mu_ps)
                musq = pa.tile([128, QC], BF16, tag="musq", name="musq")
                nc.gpsimd.tensor_mul(musq, mu_sb, mu_sb)
                stat = pa.tile([128, QC], F32, tag="stat", name="stat")
                nc.vector.tensor_sub(stat, sq_ps, musq)
                # rstd = exp(-0.5*ln(var+eps)): Ln and Exp share one ACT
                # table set with the softmax exps -> no table reloads
                nc.scalar.activation(stat, stat, AF.Ln, bias=epsb)
                rstdb = pa.tile([128, QC], BF16, tag="rstdb", name="rstdb")
                with nc.allow_low_precision(reason="rstd in bf16"):
                    nc.scalar.activation(rstdb, stat, AF.Exp, scale=-0.5)
                for cb in range(4):
                    nc.vector.tensor_sub(hT[cb], xb[cb], mu_sb)
                    nc.vector.tensor_mul(hT[cb], hT[cb], rstdb)

            def u_qk():
                hT = st["hT"]
                q_ps = ps.tile([128, QC], F32, tag="s1", name="q_ps", bufs=2)
                k_ps = ps.tile([128, QC], F32, tag="s1", name="k_ps", bufs=2)
                for cb in range(4):
                    nc.tensor.matmul(q_ps, wqs[cb], hT[cb],
                                     start=(cb == 0), stop=(cb == 3 and not qkv_bias))
                    nc.tensor.matmul(k_ps, wks[cb], hT[cb],
                                     start=(cb == 0), stop=(cb == 3 and not qkv_bias))
                if qkv_bias:
                    nc.tensor.matmul(q_ps, wqb, ones1, start=False, stop=True)
                    nc.tensor.matmul(k_ps, wkb, ones1, start=False, stop=True)
                # wq/wk columns are pre-permuted host-side so q_ps partitions
                # are [h0 lo | h1 lo | h0 hi | h1 hi]; two Pool copies (with a
                # partition shift on the upper half) land the DoubleRow layout
                nc.vector.tensor_copy(qT8[:, 0, sl], q_ps[0:64, :])
                nc.vector.tensor_copy(qT8[:, 1, sl], q_ps[64:128, :])
                nc.vector.tensor_copy(kT8[:, 0, sl], k_ps[0:64, :])
                nc.vector.tensor_copy(kT8[:, 1, sl], k_ps[64:128, :])

            def u_v():
                hT = st["hT"]
                for t4 in range(4):
                    tb = tch * 4 + t4
                    t4sl = slice(t4 * 128, (t4 + 1) * 128)
                    v_ps = ps.tile([128, 2 * HS], F32, tag="s1", name="v_ps", bufs=2)
                    for cb in range(4):
                        nc.tensor.matmul(v_ps, hT[cb][:, t4sl], wvs[cb],
                                         start=(cb == 0), stop=(cb == 3 and not qkv_bias))
                    if qkv_bias:
                        nc.tensor.matmul(v_ps, ones1[:, 0:128], wvb, start=False, stop=True)
                    for h in range(2):
                        nc.vector.tensor_copy(vst[h][:, tb, 0:HS],
                                              v_ps[:, h * HS:(h + 1) * HS])

            return [u_ln, u_qk, u_v]

        def emit_ab(tch):
            for u in ab_units(tch):
                u()

        # ========== stage C: flash attention + packed proj partial =========
        abq = []     # pending A/B units: highest priority, one per pair
        gq = []      # weight loads + stage-D units, one per pair otherwise
        tailq = []   # previous chunk's renorm/proj units: must run first so
                     # o_ps frees for this chunk's att@V accumulation

        def emit_c(c):
            qsl = slice(c * QC, (c + 1) * QC)
            nkv = 4 * (c + 1)
            npair = nkv // 2
            o_ps = [ps.tile([65, QC], F32, tag=f"o{h}", name=f"o{h}", bufs=1)
                    for h in range(2)]
            for pi in range(npair):
                if pi != 0:
                    if tailq:
                        tailq.pop(0)()
                    elif abq:
                        abq.pop(0)()
                    elif gq:
                        gq.pop(0)()
                p4 = pc_p.tile([128, 2, 2 * QC], F8, tag="p4", name="p4")
                for j2 in range(2):
                    kb = 2 * pi + j2
                    ksl = slice(kb * 128, (kb + 1) * 128)
                    diag = kb >= 4 * c
                    s_ps = ps.tile([128, 2 * QC], F32, tag="sc", name="sc", bufs=2)
                    for h in range(2):
                        hp = slice(h * 32, (h + 1) * 32)
                        nc.tensor.matmul(s_ps[:, h * QC:(h + 1) * QC],
                                         kT8[hp, :, ksl], qT8[hp, :, qsl],
                                         start=True, stop=not diag,
                                         perf_mode=DR)
                    if diag:
                        # additive causal mask (-1e6 above diagonal) folded
                        # into the PSUM accumulation: s += I^T @ mneg_j
                        j = kb - 4 * c
                        nc.tensor.matmul(s_ps[:, 0:QC], ident, tri[j][:, 0:QC],
                                         start=False, stop=True)
                        nc.tensor.matmul(s_ps[:, QC:2 * QC], ident, tri[j][:, QC:2 * QC],
                                         start=False, stop=True)
                    nc.scalar.activation(p4[:, j2, :], s_ps, AF.Exp, scale=SCL)
                if pi == 0 and tailq:
                    # previous chunk's renorm slots in between pair-0's
                    # scores/exp and its att@V, so this chunk's first av
                    # (which waits for o_ps to free) finds renorm done
                    tailq.pop(0)()
                # att @ V over the k-block pair in one fp8 DoubleRow matmul
                for h in range(2):
                    nc.tensor.matmul(o_ps[h], vst[h][:, 2 * pi:2 * pi + 2, 0:HS + 1],
                                     p4[:, :, h * QC:(h + 1) * QC],
                                     start=(pi == 0), stop=(pi == npair - 1),
                                     perf_mode=DR)
            # renorm + proj are deferred into the NEXT chunk's pair stream
            # (tailq) so their DVE->PE chain latency hides under its exps
            def t_renorm():
                on_t = pc_r.tile([128, QC], BF16, tag="on", name="on")
                for h in range(2):
                    rd = pc_r.tile([1, QC], F32, tag=f"rd{h}", name=f"rd{h}")
                    nc.vector.reciprocal(rd, o_ps[h][64:65, :])
                    rb_ps = ps.tile([64, QC], F32, tag="s1", name="rb_ps", bufs=2)
                    nc.tensor.matmul(rb_ps, ones64f, rd, start=True, stop=True)
                    rb = pc_r.tile([64, QC], F32, tag=f"rb{h}", name=f"rb{h}")
                    nc.vector.tensor_copy(rb, rb_ps)
                    nc.vector.tensor_mul(on_t[h * 64:(h + 1) * 64, :],
                                         o_ps[h][0:64, :], rb)
                st_c["on_t"] = on_t

            def t_proj():
                on_t = st_c["on_t"]
                for t4 in range(4):
                    t4sl = slice(t4 * 128, (t4 + 1) * 128)
                    pp = ps.tile([128, C], F32, tag="s1", name="pp", bufs=2)
                    nc.tensor.matmul(pp, on_t[:, t4sl], wo_sb, start=True, stop=True)
                    ppsb = pc_p.tile([128, C], BF16, tag="ppsb", name="ppsb")
                    nc.vector.tensor_copy(ppsb, pp)
                    gr = c * QC + t4 * 128
                    piece = 0 if gr < 2048 else (1 if gr < 3584 else 2)
                    r0 = gr - RS_PIECES[piece][0]
                    nc.gpsimd.dma_start(out=proj_d[piece][r0:r0 + 128, :], in_=ppsb)

            st_c = {}
            t_renorm()
            t_proj()

        # ========== stage D: LN2 -> FFN -> out for one RS piece ============
        # split into small "units" so the work can be drip-fed between
        # attention pairs (the PE/DVE queues are FIFO: one big block of
        # stage-D instructions would stall the next chunk's scores and
        # starve the exp stream)
        d_f1 = {}

        def d_ln_tb(piece, tb):
            tsl = slice(tb * 128, (tb + 1) * 128)
            rstb = pd.tile([128, C], BF16, tag="rstb", name="rstb")
            hr = (tb - PIECE_TBS[piece][0]) * 128
            nc.sync.dma_start(out=rstb, in_=rs_d[piece][hr:hr + 128, :])
            xst = pd.tile([128, C], F32, tag="xst", name="xst")
            nc.sync.dma_start(out=xst, in_=xsl_d[tsl, :])
            nc.vector.tensor_add(x2t[tb], rstb, xst)
            if has_bo:
                nc.vector.tensor_add(x2t[tb], x2t[tb], bob)
            st6 = pd.tile([128, 6], F32, tag="st6", name="st6")
            nc.vector.bn_stats(st6, x2t[tb])
            mv = pd.tile([128, 2], F32, tag="mv", name="mv")
            nc.vector.bn_aggr(mv, st6)
            vv = pd.tile([128, 1], F32, tag="vv", name="vv")
            nc.scalar.activation(vv, mv[:, 1:2], AF.Ln, bias=epsb)
            rs2 = pd.tile([128, 1], F32, tag="rs2", name="rs2")
            nc.scalar.activation(rs2, vv, AF.Exp, scale=-0.5)
            h2 = pd.tile([128, C], BF16, tag="h2", name="h2")
            nc.vector.tensor_scalar(h2, x2t[tb], mv[:, 0:1], rs2,
                                    op0=mybir.AluOpType.subtract,
                                    op1=mybir.AluOpType.mult)
            for cb in range(4):
                tp = ps.tile([128, 128], BF16, tag="s1", name="tp", bufs=2)
                nc.tensor.transpose(tp, h2[:, cb * 128:(cb + 1) * 128], ident)
                nc.vector.tensor_copy(h2T[cb][:, tsl], tp)

        def d_w1_hb(grp, hb):
            nt = len(grp)
            csl = slice(grp[0] * 128, (grp[0] + nt) * 128)
            f1 = pf.tile([128, 2 * 128], BF16, tag=f"f1_{hb}", name=f"f1_{hb}")
            d_f1[(grp[0], hb)] = f1
            hsl = slice(hb * 128, (hb + 1) * 128)
            f_ps = ps.tile([128, nt * 128], F32, tag="s1", name="f_ps", bufs=2)
            for cb in range(4):
                nc.tensor.matmul(f_ps, w1sb[cb][:, hsl], h2T[cb][:, csl],
                                 start=(cb == 0), stop=(cb == 3 and not w1_bias))
            if w1_bias:
                nc.tensor.matmul(f_ps, w1b[:, hsl], ones1[:, 0:nt * 128],
                                 start=False, stop=True)
            nc.vector.tensor_scalar_max(f1[:, 0:nt * 128], f_ps, 0.0)

        def d_w2_tb(grp, tb):
            i = tb - grp[0]
            tsl = slice(tb * 128, (tb + 1) * 128)
            fsl = slice(i * 128, (i + 1) * 128)
            ff_ps = ps.tile([128, C], F32, tag="s1", name="ff_ps", bufs=2)
            for hb in range(16):
                nc.tensor.matmul(ff_ps, d_f1[(grp[0], hb)][:, fsl], w2sb[hb],
                                 start=(hb == 0), stop=(hb == 15))
            ot = pd.tile([128, C], F32, tag="ot", name="ot")
            nc.vector.tensor_add(ot, ff_ps, x2t[tb])
            if has_b2:
                nc.vector.tensor_add(ot, ot, b2b)
            nc.sync.dma_start(out=out_d[tsl, :], in_=ot)

        def d_units(piece):
            # FFN runs per 2-tb group to bound the f1 SBUF footprint
            units = []
            tbs = PIECE_TBS[piece]
            for tb in tbs:
                units.append(lambda tb=tb: d_ln_tb(piece, tb))
            for g0 in range(0, len(tbs), 2):
                grp = tbs[g0:g0 + 2]
                for hb in range(16):
                    units.append(lambda hb=hb, grp=tuple(grp): d_w1_hb(grp, hb))
                for tb in grp:
                    units.append(lambda tb=tb, grp=tuple(grp): d_w2_tb(grp, tb))
            return units

        # ========== emission schedule ==========
        # chunk-0/1 x loads configure on the SP sequencer first (565ns per
        # DMA config); the qkv weights follow, still landing before the
        # first LN1 output reaches the q/k/v matmuls
        # startup DMA order tracks the first-exp critical chain:
        # x0 -> stats -> LN -> q/k -> scores(kb0, diagonal needs tri)
        xpre = {}
        for tch in (0, 1):
            xpre[tch] = [pa.tile([128, QC], BF16, tag=f"xb{cb}", name=f"xb{cb}")
                         for cb in range(4)]
        # chunks 4-7's x tiles persist so their loads can be issued before
        # the first ReduceScatter blocks the SP queue
        for tch in range(4, NQC):
            xpre[tch] = [sing.tile([128, QC], BF16, tag=f"xp{tch}_{cb}",
                                   name=f"xp{tch}_{cb}") for cb in range(4)]
        for cb in range(4):
            nc.sync.dma_start(out=xpre[0][cb], in_=xT_d[cb * 128:(cb + 1) * 128, 0:QC])
        for cb in range(4):
            nc.sync.dma_start(out=wqs[cb], in_=wq2_d[cb * 128:(cb + 1) * 128, :])
            nc.sync.dma_start(out=wks[cb], in_=wk2_d[cb * 128:(cb + 1) * 128, :])
        nc.sync.dma_start(out=tri[0], in_=tri_d[0])
        nc.sync.dma_start(out=ident, in_=ident_d[:])
        for j in range(1, 4):
            nc.sync.dma_start(out=tri[j], in_=tri_d[j])
        for cb in range(4):
            nc.sync.dma_start(out=xpre[1][cb], in_=xT_d[cb * 128:(cb + 1) * 128, QC:2 * QC])
        for cb in range(4):
            nc.sync.dma_start(out=wvs[cb], in_=wv2_d[cb * 128:(cb + 1) * 128, :])
        nc.sync.dma_start(out=wo_sb, in_=wo2_d[:])
        emit_ab(0)
        emit_ab(1)
        if has_bo:
            dram_row_bcast(bob, bias_d[0:1, :])
        if has_b2:
            dram_row_bcast(b2b, bias_d[1:2, :])
        # heavy FFN weights: deferred into the pair stream via gq so their
        # DMA configs never crowd the SP queue ahead of attention loads
        wq_port = {"sp": nc.sync, "act": nc.scalar, "pool": nc.gpsimd,
                   "dve": nc.vector}[os.environ.get("K_WQ", "sp")]

        def wload(dst, src):
            return lambda: wq_port.dma_start(out=dst, in_=src)

        for cb in range(4):
            gq.append(wload(w1sb[cb], w1_d[cb * 128:(cb + 1) * 128, :]))
        if w1_bias:
            gq.append(wload(w1b, w1_d[C:C + 1, :]))
        for hb in range(16):
            gq.append(wload(w2sb[hb], w2_d[hb * 128:(hb + 1) * 128, :]))

        # in-order q-chunks.  A/B units for chunk c+2 and stage-D units for
        # already-reduced RS pieces are drip-fed one per attention pair, so
        # the FIFO engine queues never hold a long block that would starve
        # the exp stream.  RS piece 0 (rows 0-2047) fires after C3, piece 1
        # (2048-3583) after C6, the small tail piece after C7.
        for c in range(NQC):
            if c + 2 < NQC:
                abq.extend(ab_units(c + 2))
            emit_c(c)
            if c == 1:
                for tch in range(4, NQC):
                    sl = slice(tch * QC, (tch + 1) * QC)
                    for cb in range(4):
                        nc.sync.dma_start(out=xpre[tch][cb],
                                          in_=xT_d[cb * 128:(cb + 1) * 128, sl])
            elif c == 3:
                # flush this chunk's renorm/proj so the RS (whose deps are
                # tracked by emission order) can be emitted now
                while tailq:
                    tailq.pop(0)()
                cc_rs(0)
            elif c == 4:
                # piece-0 D work becomes dependency-ready at ~82us (RS0
                # done); feed it into the C5/C6 pair stream
                gq.extend(d_units(0))
            elif c == 6:
                while tailq:
                    tailq.pop(0)()
                cc_rs(1)
        # leftover piece-0 units plus all of piece 1: piece 1's D runs
        # during the tail collective's window
        while tailq:
            tailq.pop(0)()
        for u in gq:
            u()
        gq.clear()
        for u in d_units(1):
            u()
        cc_rs(2)
        with tc.tile_wait_until(fence_off + float(os.environ.get("K_F2", "0.180"))):
            for u in d_units(2):
                u()


_NC_CACHE = {}


def _get_nc(flags):
    if flags not in _NC_CACHE:
        _NC_CACHE[flags] = build_nc(*flags)
    return _NC_CACHE[flags]


def make_in_maps(x, wq, wk, wv, wo, bo, w1, b1, w2, b2, g1, be1, g2, be2):
    x = np.asarray(x, np.float32)
    f32 = lambda a: np.ascontiguousarray(np.asarray(a, np.float32))
    wq, wk, wv, wo, w1, w2 = map(f32, (wq, wk, wv, wo, w1, w2))
    bo, b1, b2, g1, be1, g2, be2 = map(f32, (bo, b1, b2, g1, be1, g2, be2))

    # fold LN affine into the consuming matmuls
    wq_s = g1[None, :, None] * wq            # [H, C, HS]
    wk_s = g1[None, :, None] * wk
    wv_s = g1[None, :, None] * wv
    cq = np.einsum("c,hcd->hd", be1, wq)     # [H, HS]
    ck = np.einsum("c,hcd->hd", be1, wk)
    cv = np.einsum("c,hcd->hd", be1, wv)
    w1_s = g2[:, None] * w1                  # [C, FF]
    c1 = b1 + be2 @ w1                       # [FF]

    qkv_bias = bool(np.any(cq) or np.any(ck) or np.any(cv))
    w1_bias = bool(np.any(c1))
    has_bo = bool(np.any(bo))
    has_b2 = bool(np.any(b2))
    flags = (qkv_bias, w1_bias, has_bo, has_b2)

    w1f = np.concatenate([w1_s, c1[None, :]], 0).astype(NPBF16)   # [513, FF]
    w2f = w2.astype(NPBF16)
    biasv = np.stack([bo, b2]).astype(np.float32)                 # [2, C]

    # causal masks (additive, pre-exp) for the 4 diagonal 128-k blocks of a
    # q-chunk, in S^T layout, doubled for the 2-head tiles
    k_in = np.arange(128)[:, None]
    q_in = np.arange(QC)[None, :]
    tri = np.stack([
        np.where(j * 128 + k_in <= q_in, 0.0, -1.0e6) for j in range(4)
    ])
    tri = np.tile(tri, (1, 1, 2)).astype(NPBF16)                  # [4, 128, 2*QC]

    in_maps = []
    for r in range(N_CORES):
        b, hp = r // 4, r % 4
        h0 = 2 * hp
        s = r % 4
        # q/k columns permuted to [h0 lo32 | h1 lo32 | h0 hi32 | h1 hi32] so
        # q_ps/k_ps partitions match the DoubleRow scores layout directly
        def qperm(w_h0, w_h1, c_h0, c_h1):
            wcat = np.concatenate([w_h0[:, :32], w_h1[:, :32],
                                   w_h0[:, 32:], w_h1[:, 32:]], 1)
            ccat = np.concatenate([c_h0[:32], c_h1[:32], c_h0[32:], c_h1[32:]])
            return np.concatenate([wcat, ccat[None, :]], 0).astype(NPBF16)

        wq2 = qperm(wq_s[h0], wq_s[h0 + 1], cq[h0], cq[h0 + 1])
        wk2 = qperm(wk_s[h0], wk_s[h0 + 1], ck[h0], ck[h0 + 1])
        wv2 = np.concatenate([
            np.concatenate([wv_s[h0], wv_s[h0 + 1]], 1),
            np.concatenate([cv[h0], cv[h0 + 1]])[None, :]], 0).astype(NPBF16)
        wo2 = wo[h0 * HS:(h0 + 2) * HS, :].astype(NPBF16)         # [128, C]
        x_sl = np.concatenate(
            [x[b, g0:g0 + ln] for g0, ln in owned_slices(s)], 0)
        in_maps.append({
            "xT": np.ascontiguousarray(x[b].T).astype(NPBF16),
            "x_sl": np.ascontiguousarray(x_sl),
            "wq2": wq2, "wk2": wk2, "wv2": wv2, "wo2": wo2,
            "w1f": w1f, "w2f": w2f, "biasv": biasv, "trimask": tri,
            "identm": np.eye(128, dtype=np.float32).astype(NPBF16),
        })
    return in_maps, flags


def run_spmd(in_maps, flags, **kw):
    from concourse.bass_utils import run_bass_kernel_spmd
    nc = _get_nc(flags)
    return run_bass_kernel_spmd(nc, in_maps, list(range(N_CORES)), **kw)


def kernel(**inputs):
    in_maps, flags = make_in_maps(**inputs)
    res = run_spmd(in_maps, flags).results
    return assemble([res[r]["out"] for r in range(N_CORES)])
